# revision 36
# baseline (speedup 1.0000x reference)
"""Distributed TRN2 Bass kernel for NSA-style sparse attention.

Problem: b=1, s=2048, d=2048, 16 heads x 128 dim, f32.
  q/k/v = x @ w{q,k,v}.T ; interleaved RoPE on q,k ;
  compressed KV = mean-pool of 16 post-RoPE tokens ;
  joint softmax over [causal compressed blocks ; 256-token sliding window] ;
  out = (p @ [cv;v]) @ wo.T

Sharding: 2 heads per core (column-parallel wq/wk/wv), x replicated.
Output projection via AllToAll, chunked into 5 row pieces; collectives
and the wo matmuls overlap the attention of later chunks. Chunk 3 runs
both heads' projections first, then attention tiles row-major across
heads so piece 3 (rows 1536:1792) posts mid-chunk and piece 4 right
after the last tile; proj_slab(0) then fills the PE while the tail
collectives run.

Attention computes scores transposed ([kv, q]) so the exp output feeds
the PV matmul directly as the stationary operand. The compressed-block
scores land in the same [128,512] PSUM tile as the window scores, so a
single scalar_tensor_tensor applies scale+mask from a combined table.
The softmax denominator comes from a ones-column appended to v.

Startup DMAs are issued fine-grained in exact consumption order
(wq quarter, x0 halves, ... wk, wv) with the rope/mask constants gated
behind x0's third quarter so they don't steal critical-path bandwidth.
wo streams in during chunk 2 (gated on chunk-2 k-rope) when the rings
are idle.

Precision: matmul operands bf16 (f32 PSUM accumulation), softmax in
f32; rope multiplies in bf16.
"""
import sys, os, types

sys.path.insert(0, "/opt/trn_rl_repo")
import numpy as np

S = 2048        # sequence length
D = 2048        # model dim
H = 16          # heads
DH = 128        # head dim
RATIO = 16      # compress ratio
WINDOW = 256    # sliding window
NBLK = S // RATIO          # 128 compressed blocks
ROPE_BASE = 10000.0
NCORES = 8
HPC = H // NCORES          # 2 heads per core
CHUNK = 512                # s-columns per pipeline step
NCHUNK = S // CHUNK        # 4
KT = D // 128              # 16 contraction tiles
NEG = -1e30
VB = DH + 1                # v row block width (ones column at DH)

# A2A pieces: (row0, row1) over the s axis; per-core slab column base in bp
PIECES = [(0, 512), (512, 1024), (1024, 1536), (1536, 1792),
          (1792, 1920), (1920, 2048)]
PIECE_BASE = [0, 64, 128, 192, 224, 240]


def _setup_ntff_hook():
    try:
        import antenv
        if "antenv.axon_hooks" not in sys.modules:
            m = types.ModuleType("antenv.axon_hooks")
            m._hook = None
            m.set_axon_ntff_profile_hook = lambda h: setattr(m, "_hook", h)
            m.get_axon_ntff_profile_hook = lambda: m._hook
            sys.modules["antenv.axon_hooks"] = m
            antenv.axon_hooks = m
        if "/root/.axon_site" not in sys.path:
            sys.path.insert(0, "/root/.axon_site")
        from trn_agent_boot.trn_boot import _ntff_profile_via_ctypes
        hook = _ntff_profile_via_ctypes("/opt/axon/libaxon_pjrt.so")
        sys.modules["antenv.axon_hooks"].set_axon_ntff_profile_hook(hook)
    except Exception:
        pass


def build():
    import concourse.bass as bass
    import concourse.mybir as mybir
    from concourse import bacc, tile
    from concourse.masks import make_identity

    F32 = mybir.dt.float32
    BF16 = mybir.dt.bfloat16
    Alu = mybir.AluOpType
    Act = mybir.ActivationFunctionType
    AX = mybir.AxisListType

    nc = bacc.Bacc(None, target_bir_lowering=False, debug=False)

    xt_e = nc.declare_dram_parameter("xt", [NCHUNK, 128, KT * CHUNK], BF16,
                                     isOutput=False)
    wqt_e = nc.declare_dram_parameter("wqt", [128, KT * 256], BF16, isOutput=False)
    wkt_e = nc.declare_dram_parameter("wkt", [128, KT * 256], BF16, isOutput=False)
    wvt_e = nc.declare_dram_parameter("wvt", [128, KT * 256], BF16, isOutput=False)
    wot_e = nc.declare_dram_parameter("wot", [8, 128, 8 * 512], BF16,
                                      isOutput=False)
    cos_e = nc.declare_dram_parameter("cos", [DH, S], BF16, isOutput=False)
    sins_e = nc.declare_dram_parameter("sins", [DH, S], BF16, isOutput=False)
    swap_e = nc.declare_dram_parameter("swapm", [DH, DH], BF16, isOutput=False)
    out_e = nc.declare_dram_parameter("out", [S // NCORES, D], BF16,
                                     isOutput=True)

    scale = float(DH) ** -0.5
    Q = 1024   # columns in a quarter of a projection weight tile

    with tile.TileContext(nc) as tc:
        with (
            tc.tile_pool(name="const", bufs=1) as constp,
            tc.tile_pool(name="wpool", bufs=1) as wpool,
            tc.tile_pool(name="wopool", bufs=1) as wopool,
            tc.tile_pool(name="xstream", bufs=2) as xstream,
            tc.tile_pool(name="work", bufs=2) as work,
            tc.tile_pool(name="ps", bufs=2, space="PSUM") as ps,
            tc.tile_pool(name="dram", bufs=1, space="DRAM") as dram,
        ):
            # ---------- critical startup stream (sync queue) ----------------
            # Exact consumption order for chunk-0 head-0's sequential q/k/v
            # accumulations: wq quarter q feeds matmuls kk=4q..4q+3 together
            # with x tile q; then wk, wv halves; then x for chunks 1-3.
            wq_sb = wpool.tile([128, KT * 256], BF16, tag="wq")
            wk_sb = wpool.tile([128, KT * 256], BF16, tag="wk")
            wv_sb = wpool.tile([128, KT * 256], BF16, tag="wv")

            def x_tiles(c):
                return [xstream.tile([128, 4 * CHUNK], BF16, tag=f"xq{q}",
                                     name=f"x{c}q{q}") for q in range(4)]

            def x_dma(ts, c, q):
                nc.sync.dma_start(ts[q][:], xt_e[c][:, 4 * CHUNK * q:
                                                    4 * CHUNK * (q + 1)])

            xs = {0: x_tiles(0), 1: x_tiles(1)}
            for q in range(4):
                nc.sync.dma_start(wq_sb[:, Q * q:Q * (q + 1)],
                                  wqt_e[:, Q * q:Q * (q + 1)])
                nc.sync.dma_start(xs[0][q][:, 0:Q], xt_e[0][:, 2048 * q:
                                                            2048 * q + Q])
                nc.sync.dma_start(xs[0][q][:, Q:2048], xt_e[0][:, 2048 * q + Q:
                                                               2048 * (q + 1)])
                nc.sync.dma_start(wk_sb[:, Q * q:Q * (q + 1)],
                                  wkt_e[:, Q * q:Q * (q + 1)])
                nc.sync.dma_start(wv_sb[:, Q * q:Q * (q + 1)],
                                  wvt_e[:, Q * q:Q * (q + 1)])
            for c in (2, 3):
                xs[c] = x_tiles(c)

            # ---------- constants (gpsimd queue), gated behind x0q2 ---------
            swap_sb = constp.tile([DH, DH], BF16, tag="swap")
            cos_sb = constp.tile([DH, S], BF16, tag="cos")
            sins_sb = constp.tile([DH, S], BF16, tag="sins")
            identb = constp.tile([128, 128], BF16, tag="identb")
            maskc_sb = constp.tile([128, 16 * 512], BF16, tag="maskc")
            gate = constp.tile([1, 1], BF16, tag="gate")

            def gen_mask(tg):
                # 0/1 mask for query tile tg, generated on the idle gpsimd
                # engine (saves 2MB of HBM from the startup window):
                # cols 0:128 comp blocks vis iff blk < 8tg + (r+1)//16;
                # window tiles w=tg-2+j: j=0 anti-triangle, j=1 full,
                # j=2 causal triangle; padded tiles (w<0) fully masked
                base = 512 * tg
                comp = maskc_sb[:, base:base + 128]
                nc.gpsimd.memset(comp, 1.0)
                cv = comp.rearrange("p (rb rr) -> p rb rr", rr=16)
                # (r+1)//16 = rb + (rr==15): two disjoint affine selects
                # (only is_gt/is_ge lower on gpsimd, so conditions are
                # written as rb + 8tg(+1) - blk > 0)
                nc.gpsimd.affine_select(
                    out=cv[:, :, 0:15], in_=cv[:, :, 0:15],
                    compare_op=Alu.is_gt, fill=0.0, base=8 * tg,
                    pattern=[[1, 8], [0, 15]], channel_multiplier=-1)
                nc.gpsimd.affine_select(
                    out=cv[:, :, 15:16], in_=cv[:, :, 15:16],
                    compare_op=Alu.is_gt, fill=0.0, base=8 * tg + 1,
                    pattern=[[1, 8], [0, 1]], channel_multiplier=-1)
                win = maskc_sb[:, base + 128:base + 512]
                if tg == 0:
                    nc.gpsimd.memset(win[:, 0:256], 0.0)
                    nc.gpsimd.memset(win[:, 256:384], 1.0)
                else:
                    nc.gpsimd.memset(win, 1.0)
                    if tg == 1:
                        nc.gpsimd.memset(win[:, 0:128], 0.0)
                    else:
                        # j=0: vis iff p > r
                        nc.gpsimd.affine_select(
                            out=win[:, 0:128], in_=win[:, 0:128],
                            compare_op=Alu.is_gt, fill=0.0, base=0,
                            pattern=[[-1, 128]], channel_multiplier=1)
                # j=2: vis iff p <= r, i.e. r - p >= 0
                nc.gpsimd.affine_select(
                    out=win[:, 256:384], in_=win[:, 256:384],
                    compare_op=Alu.is_ge, fill=0.0, base=0,
                    pattern=[[1, 128]], channel_multiplier=-1)

            # RAW gate: rope tables enqueue once x0q1 landed so they don't
            # steal ring bandwidth from the critical first quarters; gate and
            # const DMAs go FIRST on the gpsimd queue so nothing delays them
            nc.gpsimd.tensor_copy(gate[0:1, 0:1], xs[0][1][0:1, 0:1])
            for t in (swap_sb, cos_sb, sins_sb):
                nc.gpsimd.tensor_tensor(t[0:1, 0:1], gate[0:1, 0:1],
                                        gate[0:1, 0:1], Alu.mult)
            nc.gpsimd.dma_start(swap_sb[:], swap_e[:])
            nc.gpsimd.dma_start(cos_sb[:], cos_e[:])
            nc.gpsimd.dma_start(sins_sb[:], sins_e[:])
            # x prefetch for chunks 1-3, gated behind the rope consts:
            # ungated they flood the ring FIFOs at t=0 and the consts then
            # crawl in behind 6MB of prefetch, stalling chunk-0's ropes
            for c in (1, 2, 3):
                for q in range(4):
                    nc.gpsimd.tensor_tensor(xs[c][q][0:1, 0:1],
                                            gate[0:1, 0:1], gate[0:1, 0:1],
                                            Alu.mult)
            for c in (1, 2, 3):
                for q in range(4):
                    x_dma(xs[c], c, q)
            make_identity(nc, identb[:])
            for tg in range(16):
                gen_mask(tg)

            # ---------- persistent per-head state ----------
            kt_full = [work.tile([DH, WINDOW + S], BF16, tag=f"ktf{h}", bufs=1,
                                 name=f"ktf{h}") for h in range(HPC)]
            # v rows with a ones column per 129-wide block; first 2 blocks pad
            vrow = [work.tile([128, (2 + S // 128) * VB], BF16, tag=f"vrow{h}",
                              bufs=1, name=f"vrow{h}") for h in range(HPC)]
            ckt = [work.tile([DH, NBLK], BF16, tag=f"ckt{h}", bufs=1,
                             name=f"ckt{h}") for h in range(HPC)]
            cvrow = [work.tile([NBLK, VB], BF16, tag=f"cvrow{h}", bufs=1,
                               name=f"cvrow{h}") for h in range(HPC)]
            cvt_acc = [work.tile([DH, NBLK], BF16, tag=f"cvt{h}", bufs=1,
                                 name=f"cvt{h}") for h in range(HPC)]
            for h in range(HPC):
                nc.vector.memset(kt_full[h][:, 0:WINDOW], 0.0)
                nc.vector.memset(vrow[h][:], 0.0)
                nc.vector.memset(
                    vrow[h][:].rearrange("p (b c) -> p b c", c=VB)[:, :, DH:],
                    1.0)
                nc.vector.memset(ckt[h][:], 0.0)
                nc.vector.memset(cvrow[h][:], 0.0)
                nc.vector.memset(cvrow[h][:, DH:], 1.0)
                nc.vector.memset(cvt_acc[h][:], 0.0)

            # wo tiles: loaded on the scalar queue once chunk 2's k-rope has
            # landed (rings are idle by then; earlier loads would compete
            # with the startup bulk stream)
            wo_tiles = [wopool.tile([128, 8 * 512], BF16, tag=f"wo{i}",
                                    name=f"wo{i}") for i in range(8)]

            # a2a bounce buffers: one tensor pair PER piece, in dest-major
            # TRANSPOSED layout [8 peers, 256 dims, rows-per-peer]
            a2a_in = [dram.tile([8, HPC * DH, (r1 - r0) // 8], BF16,
                                tag=f"a2ai{p}", name=f"a2ai{p}")
                      for p, (r0, r1) in enumerate(PIECES)]
            a2a_out = [dram.tile([8, HPC * DH, (r1 - r0) // 8], BF16,
                                 tag=f"a2ao{p}", name=f"a2ao{p}")
                       for p, (r0, r1) in enumerate(PIECES)]

            # bp: lhsT for the output projection. col layout per contraction
            # tile kk: [0:128]=pieces 0+1, [128:192]=piece2, [192:224]=p3,
            # [224:256]=p4
            bp_sb = xstream.tile([128, KT * 256], BF16, tag="bpt", bufs=1)

            def a2a_piece(p):
                nc.gpsimd.collective_compute(
                    "AllToAll", mybir.AluOpType.bypass,
                    replica_groups=[list(range(NCORES))],
                    ins=[a2a_in[p][:].opt()], outs=[a2a_out[p][:].opt()],
                )

            def piece_fill(p, engine):
                # bp[pp, 512i+256u+base+c] = a2a_out[p][i, 128u+pp, c]
                rp = (PIECES[p][1] - PIECES[p][0]) // 8
                base = PIECE_BASE[p]
                dst = bp_sb[:].rearrange("pp (i u c) -> pp i u c",
                                         i=8, c=256)[:, :, :, base:base + rp]
                srcv = a2a_out[p][:].rearrange("i (u pp) c -> pp i u c", u=2)
                engine.dma_start(dst, srcv)

            def proj_slab(m):
                # out rows [128m : 128m+128) of this core's 256-row share
                for n in range(4):
                    wo_sb, wo_sb2 = wo_tiles[2 * n], wo_tiles[2 * n + 1]
                    acc = ps.tile([128, 512], F32, tag=("sc", "ot")[n % 2],
                                  bufs=(3, 2)[n % 2], name=f"pacc{m}{n}")
                    for kk in range(KT):
                        wsb = wo_sb if kk < 8 else wo_sb2
                        nc.tensor.matmul(
                            acc[:],
                            bp_sb[:, 256 * kk + 128 * m:256 * kk + 128 * (m + 1)],
                            wsb[:, 512 * (kk % 8):512 * (kk % 8 + 1)],
                            start=(kk == 0), stop=(kk == KT - 1),
                        )
                    outsb = work.tile([128, 512], BF16, tag="outsb", bufs=2)
                    nc.vector.tensor_copy(outsb[:], acc[:])
                    eng = nc.sync if n % 2 == 0 else nc.scalar
                    eng.dma_start(
                        out_e[128 * m:128 * (m + 1), 512 * n:512 * (n + 1)],
                        outsb[:])

            # ---------- per-chunk work ----------
            def rope(acc, dest_ap, cols, nm):
                raw = work.tile([DH, CHUNK], BF16, tag="qraw", bufs=2,
                                name=f"raw{nm}")
                nc.scalar.copy(raw[:], acc[:])
                sw_ps = ps.tile([DH, CHUNK], F32, tag="sc", bufs=3,
                                name=f"sw{nm}")
                nc.tensor.matmul(sw_ps[:], swap_sb[:], raw[:],
                                 start=True, stop=True)
                t1 = work.tile([DH, CHUNK], BF16, tag="rope1", bufs=2,
                               name=f"t1{nm}")
                nc.vector.tensor_tensor(t1[:], raw[:], cos_sb[:, cols],
                                        Alu.mult)
                t2 = work.tile([DH, CHUNK], BF16, tag="rope2", bufs=2,
                               name=f"t2{nm}")
                nc.vector.tensor_tensor(t2[:], sw_ps[:], sins_sb[:, cols],
                                        Alu.mult)
                nc.vector.tensor_tensor(dest_ap, t1[:], t2[:], Alu.add)

            def head_setup(c, h):
                """q/k/v projections + rope + pooled/transposed kv state."""
                col0 = CHUNK * c
                cols = slice(col0, col0 + CHUNK)
                x_sb = xs[c]

                def xsl(kk):
                    t, r = kk // 4, kk % 4
                    return x_sb[t][:, CHUNK * r:CHUNK * (r + 1)]

                def one_mm(acc, w_sb, kk):
                    nc.tensor.matmul(
                        acc[:],
                        w_sb[:, 256 * kk + 128 * h:256 * kk + 128 * (h + 1)],
                        xsl(kk),
                        start=(kk == 0), stop=(kk == KT - 1),
                    )

                def acc_mm(w_sb, tag, nm):
                    acc = ps.tile([128, CHUNK], F32, tag="acc", bufs=3,
                                  name=nm)
                    for kk in range(KT):
                        one_mm(acc, w_sb, kk)
                    return acc

                if c == 0 and h == 0:
                    # chunk-0 h0 is fed at HBM rate: interleave q/k/v per x
                    # quarter so the PE consumes exactly at the delivery order
                    # and never outruns the stream
                    accs = [ps.tile([128, CHUNK], F32, tag="acc", bufs=3,
                                    name=f"acc{n}00") for n in "qkv"]
                    for qtr in range(4):
                        for acc, w_sb in zip(accs, (wq_sb, wk_sb, wv_sb)):
                            for kk in range(4 * qtr, 4 * qtr + 4):
                                one_mm(acc, w_sb, kk)
                    acc_q, acc_k0, acc_v0 = accs
                else:
                    acc_q = acc_mm(wq_sb, "accq", f"accq{c}{h}")
                # q
                q_sb = work.tile([DH, CHUNK], BF16, tag=f"qt{h}", bufs=1,
                                 name=f"qt{c}{h}")
                rope(acc_q, q_sb[:], cols, f"q{c}{h}")
                # k -> kt_full (post-rope), then pooled ck
                acc_k = acc_k0 if c == 0 and h == 0 else \
                    acc_mm(wk_sb, "acck", f"acck{c}{h}")
                kdst = kt_full[h][:, WINDOW + col0:WINDOW + col0 + CHUNK]
                rope(acc_k, kdst, cols, f"k{c}{h}")
                cks = work.tile([DH, CHUNK // RATIO], F32, tag="cks", bufs=2,
                                name=f"cks{c}{h}")
                nc.vector.tensor_reduce(
                    cks[:], kdst.rearrange("p (b r) -> p b r", r=RATIO),
                    AX.X, Alu.add)
                nc.vector.tensor_scalar_mul(
                    ckt[h][:, col0 // RATIO:(col0 + CHUNK) // RATIO],
                    cks[:], 1.0 / RATIO)
                # v: copy to bf16, pool cv, transpose to row-major
                acc_v = acc_v0 if c == 0 and h == 0 else \
                    acc_mm(wv_sb, "accv", f"accv{c}{h}")
                vt = work.tile([DH, CHUNK], BF16, tag="vt", bufs=2,
                               name=f"vt{c}{h}")
                nc.scalar.copy(vt[:], acc_v[:])
                cvs = work.tile([DH, CHUNK // RATIO], F32, tag="cks", bufs=2,
                                name=f"cvs{c}{h}")
                nc.vector.tensor_reduce(
                    cvs[:], vt[:].rearrange("p (b r) -> p b r", r=RATIO),
                    AX.X, Alu.add)
                nc.vector.tensor_scalar_mul(
                    cvt_acc[h][:, col0 // RATIO:(col0 + CHUNK) // RATIO],
                    cvs[:], 1.0 / RATIO)
                cv_ps = ps.tile([NBLK, DH], BF16, tag="ot", bufs=2,
                                name=f"cvp{c}{h}")
                nc.tensor.transpose(cv_ps[:], cvt_acc[h][:], identb[:])
                nc.scalar.copy(cvrow[h][:, 0:DH], cv_ps[:])
                vtr_ps = ps.tile([128, CHUNK], BF16, tag="ot", bufs=2,
                                 name=f"vtp{c}{h}")
                for tt in range(CHUNK // 128):
                    nc.tensor.transpose(vtr_ps[:, 128 * tt:128 * (tt + 1)],
                                        vt[:, 128 * tt:128 * (tt + 1)],
                                        identb[:])
                st0 = CHUNK // 128 * c  # first raw s-tile of this chunk
                for tt in range(CHUNK // 128):
                    nc.scalar.copy(
                        vrow[h][:, (st0 + 2 + tt) * VB:
                                (st0 + 2 + tt) * VB + DH],
                        vtr_ps[:, 128 * tt:128 * (tt + 1)])
                return q_sb

            def attn_tile(c, h, lt, q_sb):
                tg = CHUNK // 128 * c + lt   # global query tile
                qs = q_sb[:, 128 * lt:128 * (lt + 1)]
                # scores transposed [kv, q]: cols 0:128 = compressed blocks,
                # 128:512 = 3 window kv tiles
                s_ps = ps.tile([128, 512], F32, tag="sc", bufs=3,
                               name=f"sps{c}{h}{lt}")
                nc.tensor.matmul(s_ps[:, 0:128], ckt[h][:], qs,
                                 start=True, stop=True)
                for j in range(3):
                    nc.tensor.matmul(
                        s_ps[:, 128 * (j + 1):128 * (j + 2)],
                        kt_full[h][:, 128 * (tg + j):128 * (tg + j) + 128],
                        qs, start=True, stop=True)
                # p = exp(s*scale) ⊙ mask01: exp straight from PSUM (scale
                # folded into the activation), multiplicative 0/1 mask on the
                # DVE in bf16 — one hop shorter than additive-mask-then-exp,
                # and the masked entries never reach the ones-column denom
                p_r = work.tile([128, 512], BF16, tag="praw", bufs=4,
                                name=f"praw{c}{h}{lt}")
                nc.scalar.activation(p_r[:], s_ps[:], Act.Exp,
                                     bias=0.0, scale=scale)
                p_t = work.tile([128, 512], BF16, tag="psb", bufs=4,
                                name=f"psb{c}{h}{lt}")
                if c == 3 and h == 1 and lt == 3:
                    state["pt"] = p_t
                nc.vector.tensor_tensor(p_t[:], p_r[:],
                                        maskc_sb[:, 512 * tg:512 * (tg + 1)],
                                        Alu.mult)
                # PV: lhsT = p blocks, rhs = v rows + ones col
                o_ps = ps.tile([128, VB], F32, tag="ot", bufs=2,
                               name=f"ops{c}{h}{lt}")
                nc.tensor.matmul(o_ps[:], p_t[:, 0:128], cvrow[h][:],
                                 start=True, stop=False)
                for j in range(3):
                    w = tg - 2 + j  # raw s-tile; vrow block w+2
                    nc.tensor.matmul(
                        o_ps[:], p_t[:, 128 * (j + 1):128 * (j + 2)],
                        vrow[h][:, (w + 2) * VB:(w + 3) * VB],
                        start=False, stop=(j == 2))
                rl = work.tile([128, 1], F32, tag="stat", bufs=16,
                               name=f"rl{c}{h}{lt}")
                nc.vector.reciprocal(rl[:], o_ps[:, DH:DH + 1])
                osb = work.tile([128, DH], BF16, tag="osb", bufs=4,
                                name=f"osb{c}{h}{lt}")
                nc.vector.tensor_scalar_mul(osb[:], o_ps[:, 0:DH], rl[:])
                osbt_ps = ps.tile([128, DH], BF16, tag="ot", bufs=2,
                                  name=f"otp{c}{h}{lt}")
                nc.tensor.transpose(osbt_ps[:], osb[:], identb[:])
                osbt = work.tile([128, DH], BF16, tag="osbt", bufs=4,
                                 name=f"osbt{c}{h}{lt}")
                nc.vector.tensor_copy(osbt[:], osbt_ps[:])
                if c == 3 and h == 1:
                    # gate proj_slab(0)'s n=lt accumulation chain on this
                    # tile's output: each chain's start-matmul reads the
                    # wo cell written here, so the in-order PE stream
                    # interleaves slab-0 chains between the PV matmuls
                    # instead of hoisting the whole slab ahead of them
                    for i in (2 * lt, 2 * lt + 1):
                        nc.vector.scalar_tensor_tensor(
                            wo_tiles[i][0:1, 0:1], osb[0:1, 0:1], 0.0,
                            wo_tiles[i][0:1, 0:1], Alu.mult, Alu.add)
                if c == 1:
                    # stream one 1MB wo slab per chunk-1 tile (gated on this
                    # tile's output so the scheduler can't hoist the load)
                    i = 2 * lt + h
                    nc.vector.tensor_copy(wo_tiles[i][0:1, 0:1],
                                          osb[0:1, 0:1])
                    nc.sync.dma_start(wo_tiles[i][:], wot_e[i])
                pp = tg // 4 if tg < 12 else (3 if tg < 14 else tg - 10)
                rp = (PIECES[pp][1] - PIECES[pp][0]) // NCORES
                row0 = 128 * tg - PIECES[pp][0]
                # one dma_start per tile (partition dim stays first on
                # both sides)
                j0, nj = row0 // rp, 128 // rp
                nc.scalar.dma_start(
                    a2a_in[pp][j0:j0 + nj, 128 * h:128 * (h + 1), :]
                    .rearrange("j d c -> d j c"),
                    osbt[:].rearrange("d (j c) -> d j c", j=nj))

            # ---------- main pipeline over s-chunks ----------
            state = {}
            for c in range(NCHUNK):
                # (wo loads are issued inside attn_tile, one 1MB slab per
                # chunk-1 attention tile, so the 8.4MB never lands as one
                # burst: a burst saturates the chip-shared HBM and queues
                # ahead of the a2a_in writes in the ring FIFOs, which delays
                # every core's arrival at the piece collectives)
                q_sbs = [head_setup(c, h) for h in range(HPC)]
                for lt in range(CHUNK // 128):
                    for h in range(HPC):
                        attn_tile(c, h, lt, q_sbs[h])
                    if c == 3 and lt == 1:
                        a2a_piece(3)   # rows 1536:1792 done mid-chunk
                    if c == 3 and lt == 2:
                        a2a_piece(4)   # rows 1792:1920
                # ---------- overlapped collectives + projection ----------
                # fills live on the gpsimd queue, emitted just before the
                # NEXT collective: the gpsimd sequencer blocks on each
                # collective's completion anyway, so fill p triggers the
                # moment piece p is done without blocking anything else
                if c == 0:
                    a2a_piece(0)
                elif c == 1:
                    a2a_piece(1)
                elif c == 2:
                    a2a_piece(2)
                    piece_fill(0, nc.sync)
                    piece_fill(1, nc.sync)
                elif c == 3:
                    a2a_piece(5)       # tail piece (rows 1920:2048)
                    piece_fill(2, nc.sync)
                    piece_fill(3, nc.sync)
                    piece_fill(4, nc.sync)
                    piece_fill(5, nc.sync)
                    # warm-keeper: dead accumulation gated on the last
                    # tile's p so the PE clock stays at full speed through
                    # the tail collective window (slab1 then starts hot)
                    wacc = ps.tile([128, 512], F32, tag="acc", bufs=3,
                                   name="warm")
                    for i in range(28):
                        nc.tensor.matmul(wacc[:], identb[:], state["pt"][:],
                                         start=(i == 0), stop=(i == 27))
                    proj_slab(0)       # interleaved with chunk-3 PVs
                    proj_slab(1)
    return nc


def _host_inputs(x, wq, wk, wv, wo):
    """Build per-core input maps (numpy)."""
    import ml_dtypes
    BF = ml_dtypes.bfloat16
    xT = x.reshape(S, D).T.astype(BF)          # [D, S]
    xtile = np.ascontiguousarray(
        xT.reshape(KT, 128, NCHUNK, CHUNK).transpose(2, 1, 0, 3)
        .reshape(NCHUNK, 128, KT * CHUNK))
    woT = wo.T.astype(BF)                      # [D, D]
    wotile = np.ascontiguousarray(
        woT.reshape(2, 8, 128, 4, 512).transpose(3, 0, 2, 1, 4)
        .reshape(8, 128, 8 * 512))

    def wtile(w, rows):
        wT = w[rows, :].T.astype(BF)           # [D, 256]
        return np.ascontiguousarray(
            wT.reshape(KT, 128, 256).transpose(1, 0, 2).reshape(128, KT * 256))

    inv = 1.0 / (ROPE_BASE ** (np.arange(0, DH, 2, dtype=np.float32) / DH))
    theta = np.outer(np.arange(S, dtype=np.float32), inv)  # [S, 64]
    cos = np.cos(theta).T  # [64, S]
    sin = np.sin(theta).T
    COS = np.empty((DH, S), np.float32)
    SINS = np.empty((DH, S), np.float32)
    COS[0::2] = cos
    COS[1::2] = cos
    SINS[0::2] = -sin
    SINS[1::2] = sin

    SWAP = np.zeros((DH, DH), np.float32)
    for t in range(DH // 2):
        SWAP[2 * t + 1, 2 * t] = 1.0
        SWAP[2 * t, 2 * t + 1] = 1.0

    in_maps = []
    for cid in range(NCORES):
        rows = slice(256 * cid, 256 * (cid + 1))
        in_maps.append({
            "xt": xtile,
            "wqt": wtile(wq, rows),
            "wkt": wtile(wk, rows),
            "wvt": wtile(wv, rows),
            "wot": wotile,
            "cos": COS.astype(BF),
            "sins": SINS.astype(BF),
            "swapm": SWAP.astype(BF),
        })
    return in_maps


_CACHE = {}
LAST_EXEC_NS = None
LAST_RES = None


def kernel(x, wq, wk, wv, wo):
    _setup_ntff_hook()
    from concourse.bass_utils import run_bass_kernel_spmd

    if "nc" not in _CACHE:
        ncb = build()
        if not ncb.is_finalized():
            ncb.finalize()
        _CACHE["nc"] = ncb
    ncb = _CACHE["nc"]

    in_maps = _host_inputs(np.asarray(x), np.asarray(wq), np.asarray(wk),
                           np.asarray(wv), np.asarray(wo))
    trace = bool(os.environ.get("KERNEL_TRACE"))
    res = run_bass_kernel_spmd(ncb, in_maps, list(range(NCORES)), trace=trace)
    globals()["LAST_EXEC_NS"] = res.exec_time_ns
    globals()["LAST_RES"] = res
    # Each core's 256 out rows: piece p contributes rp=(r1-r0)/8 local
    # rows mapping to global rows r0 + rp*cid + l.
    out = np.empty((S, D), np.float32)
    for cid in range(NCORES):
        o = np.asarray(res.results[cid]["out"], dtype=np.float32)
        off = 0
        for p, (r0, r1) in enumerate(PIECES):
            rp = (r1 - r0) // NCORES
            out[r0 + rp * cid:r0 + rp * (cid + 1)] = o[off:off + rp]
            off += rp
    return out.reshape(1, S, D)


if __name__ == "__main__":
    rng = np.random.default_rng(0)
    x = rng.standard_normal((1, S, D), dtype=np.float32)
    wq = rng.standard_normal((D, D), dtype=np.float32) * D ** -0.5
    wk = rng.standard_normal((D, D), dtype=np.float32) * D ** -0.5
    wv = rng.standard_normal((D, D), dtype=np.float32) * D ** -0.5
    wo = rng.standard_normal((D, D), dtype=np.float32) * D ** -0.5
    out = kernel(x=x, wq=wq, wk=wk, wv=wv, wo=wo)
    print("out", out.shape, out.dtype, np.abs(out).mean())


# revision 38
# speedup vs baseline: 1.0579x; 1.0579x over previous
"""Distributed TRN2 Bass kernel for NSA-style sparse attention.

Problem: b=1, s=2048, d=2048, 16 heads x 128 dim, f32.
  q/k/v = x @ w{q,k,v}.T ; interleaved RoPE on q,k ;
  compressed KV = mean-pool of 16 post-RoPE tokens ;
  joint softmax over [causal compressed blocks ; 256-token sliding window] ;
  out = (p @ [cv;v]) @ wo.T

Sharding: 2 heads per core (column-parallel wq/wk/wv), x replicated.
Output projection via AllToAll, chunked into 5 row pieces; collectives
and the wo matmuls overlap the attention of later chunks. Chunk 3 runs
both heads' projections first, then attention tiles row-major across
heads so piece 3 (rows 1536:1792) posts mid-chunk and piece 4 right
after the last tile; proj_slab(0) then fills the PE while the tail
collectives run.

Attention computes scores transposed ([kv, q]) so the exp output feeds
the PV matmul directly as the stationary operand. The compressed-block
scores land in the same [128,512] PSUM tile as the window scores, so a
single scalar_tensor_tensor applies scale+mask from a combined table.
The softmax denominator comes from a ones-column appended to v.

Startup DMAs are issued fine-grained in exact consumption order
(wq quarter, x0 halves, ... wk, wv) with the rope/mask constants gated
behind x0's third quarter so they don't steal critical-path bandwidth.
wo streams in during chunk 2 (gated on chunk-2 k-rope) when the rings
are idle.

Precision: matmul operands bf16 (f32 PSUM accumulation), softmax in
f32; rope multiplies in bf16.
"""
import sys, os, types

sys.path.insert(0, "/opt/trn_rl_repo")
import numpy as np

S = 2048        # sequence length
D = 2048        # model dim
H = 16          # heads
DH = 128        # head dim
RATIO = 16      # compress ratio
WINDOW = 256    # sliding window
NBLK = S // RATIO          # 128 compressed blocks
ROPE_BASE = 10000.0
NCORES = 8
HPC = H // NCORES          # 2 heads per core
CHUNK = 512                # s-columns per pipeline step
NCHUNK = S // CHUNK        # 4
KT = D // 128              # 16 contraction tiles
NEG = -1e30
VB = DH + 1                # v row block width (ones column at DH)

# A2A pieces: (row0, row1) over the s axis; per-core slab column base in bp
PIECES = [(0, 512), (512, 1024), (1024, 1536), (1536, 1792), (1792, 2048)]
PIECE_BASE = [0, 64, 128, 192, 224]


def _setup_ntff_hook():
    try:
        import antenv
        if "antenv.axon_hooks" not in sys.modules:
            m = types.ModuleType("antenv.axon_hooks")
            m._hook = None
            m.set_axon_ntff_profile_hook = lambda h: setattr(m, "_hook", h)
            m.get_axon_ntff_profile_hook = lambda: m._hook
            sys.modules["antenv.axon_hooks"] = m
            antenv.axon_hooks = m
        if "/root/.axon_site" not in sys.path:
            sys.path.insert(0, "/root/.axon_site")
        from trn_agent_boot.trn_boot import _ntff_profile_via_ctypes
        hook = _ntff_profile_via_ctypes("/opt/axon/libaxon_pjrt.so")
        sys.modules["antenv.axon_hooks"].set_axon_ntff_profile_hook(hook)
    except Exception:
        pass


def build():
    import concourse.bass as bass
    import concourse.mybir as mybir
    from concourse import bacc, tile
    from concourse.masks import make_identity

    F32 = mybir.dt.float32
    BF16 = mybir.dt.bfloat16
    Alu = mybir.AluOpType
    Act = mybir.ActivationFunctionType
    AX = mybir.AxisListType

    nc = bacc.Bacc(None, target_bir_lowering=False, debug=False)

    xt_e = nc.declare_dram_parameter("xt", [NCHUNK, 128, KT * CHUNK], BF16,
                                     isOutput=False)
    wqt_e = nc.declare_dram_parameter("wqt", [128, KT * 256], BF16, isOutput=False)
    wkt_e = nc.declare_dram_parameter("wkt", [128, KT * 256], BF16, isOutput=False)
    wvt_e = nc.declare_dram_parameter("wvt", [128, KT * 256], BF16, isOutput=False)
    wot_e = nc.declare_dram_parameter("wot", [8, 128, 8 * 512], BF16,
                                      isOutput=False)
    cos_e = nc.declare_dram_parameter("cos", [DH, S], BF16, isOutput=False)
    sins_e = nc.declare_dram_parameter("sins", [DH, S], BF16, isOutput=False)
    swap_e = nc.declare_dram_parameter("swapm", [DH, DH], BF16, isOutput=False)
    out_e = nc.declare_dram_parameter("out", [S // NCORES, D], BF16,
                                     isOutput=True)

    scale = float(DH) ** -0.5
    Q = 1024   # columns in a quarter of a projection weight tile

    with tile.TileContext(nc) as tc:
        with (
            tc.tile_pool(name="const", bufs=1) as constp,
            tc.tile_pool(name="wpool", bufs=1) as wpool,
            tc.tile_pool(name="wopool", bufs=1) as wopool,
            tc.tile_pool(name="xstream", bufs=2) as xstream,
            tc.tile_pool(name="work", bufs=2) as work,
            tc.tile_pool(name="ps", bufs=2, space="PSUM") as ps,
            tc.tile_pool(name="dram", bufs=1, space="DRAM") as dram,
        ):
            # ---------- critical startup stream (sync queue) ----------------
            # Exact consumption order for chunk-0 head-0's sequential q/k/v
            # accumulations: wq quarter q feeds matmuls kk=4q..4q+3 together
            # with x tile q; then wk, wv halves; then x for chunks 1-3.
            wq_sb = wpool.tile([128, KT * 256], BF16, tag="wq")
            wk_sb = wpool.tile([128, KT * 256], BF16, tag="wk")
            wv_sb = wpool.tile([128, KT * 256], BF16, tag="wv")

            def x_tiles(c):
                return [xstream.tile([128, 4 * CHUNK], BF16, tag=f"xq{q}",
                                     name=f"x{c}q{q}") for q in range(4)]

            def x_dma(ts, c, q):
                nc.sync.dma_start(ts[q][:], xt_e[c][:, 4 * CHUNK * q:
                                                    4 * CHUNK * (q + 1)])

            xs = {0: x_tiles(0), 1: x_tiles(1)}
            for q in range(4):
                nc.sync.dma_start(wq_sb[:, Q * q:Q * (q + 1)],
                                  wqt_e[:, Q * q:Q * (q + 1)])
                nc.sync.dma_start(xs[0][q][:, 0:Q], xt_e[0][:, 2048 * q:
                                                            2048 * q + Q])
                nc.sync.dma_start(xs[0][q][:, Q:2048], xt_e[0][:, 2048 * q + Q:
                                                               2048 * (q + 1)])
                nc.sync.dma_start(wk_sb[:, Q * q:Q * (q + 1)],
                                  wkt_e[:, Q * q:Q * (q + 1)])
                nc.sync.dma_start(wv_sb[:, Q * q:Q * (q + 1)],
                                  wvt_e[:, Q * q:Q * (q + 1)])
            for c in (2, 3):
                xs[c] = x_tiles(c)

            # ---------- constants (gpsimd queue), gated behind x0q2 ---------
            swap_sb = constp.tile([DH, DH], BF16, tag="swap")
            cos_sb = constp.tile([DH, S], BF16, tag="cos")
            sins_sb = constp.tile([DH, S], BF16, tag="sins")
            identb = constp.tile([128, 128], BF16, tag="identb")
            maskc_sb = constp.tile([128, 16 * 512], BF16, tag="maskc")
            gate = constp.tile([1, 1], BF16, tag="gate")

            def gen_mask(tg):
                # 0/1 mask for query tile tg, generated on the idle gpsimd
                # engine (saves 2MB of HBM from the startup window):
                # cols 0:128 comp blocks vis iff blk < 8tg + (r+1)//16;
                # window tiles w=tg-2+j: j=0 anti-triangle, j=1 full,
                # j=2 causal triangle; padded tiles (w<0) fully masked
                base = 512 * tg
                comp = maskc_sb[:, base:base + 128]
                nc.gpsimd.memset(comp, 1.0)
                cv = comp.rearrange("p (rb rr) -> p rb rr", rr=16)
                # (r+1)//16 = rb + (rr==15): two disjoint affine selects
                # (only is_gt/is_ge lower on gpsimd, so conditions are
                # written as rb + 8tg(+1) - blk > 0)
                nc.gpsimd.affine_select(
                    out=cv[:, :, 0:15], in_=cv[:, :, 0:15],
                    compare_op=Alu.is_gt, fill=0.0, base=8 * tg,
                    pattern=[[1, 8], [0, 15]], channel_multiplier=-1)
                nc.gpsimd.affine_select(
                    out=cv[:, :, 15:16], in_=cv[:, :, 15:16],
                    compare_op=Alu.is_gt, fill=0.0, base=8 * tg + 1,
                    pattern=[[1, 8], [0, 1]], channel_multiplier=-1)
                win = maskc_sb[:, base + 128:base + 512]
                if tg == 0:
                    nc.gpsimd.memset(win[:, 0:256], 0.0)
                    nc.gpsimd.memset(win[:, 256:384], 1.0)
                else:
                    nc.gpsimd.memset(win, 1.0)
                    if tg == 1:
                        nc.gpsimd.memset(win[:, 0:128], 0.0)
                    else:
                        # j=0: vis iff p > r
                        nc.gpsimd.affine_select(
                            out=win[:, 0:128], in_=win[:, 0:128],
                            compare_op=Alu.is_gt, fill=0.0, base=0,
                            pattern=[[-1, 128]], channel_multiplier=1)
                # j=2: vis iff p <= r, i.e. r - p >= 0
                nc.gpsimd.affine_select(
                    out=win[:, 256:384], in_=win[:, 256:384],
                    compare_op=Alu.is_ge, fill=0.0, base=0,
                    pattern=[[1, 128]], channel_multiplier=-1)

            # RAW gate: rope tables enqueue once x0q1 landed so they don't
            # steal ring bandwidth from the critical first quarters; gate and
            # const DMAs go FIRST on the gpsimd queue so nothing delays them
            nc.gpsimd.tensor_copy(gate[0:1, 0:1], xs[0][1][0:1, 0:1])
            for t in (swap_sb, cos_sb, sins_sb):
                nc.gpsimd.tensor_tensor(t[0:1, 0:1], gate[0:1, 0:1],
                                        gate[0:1, 0:1], Alu.mult)
            nc.gpsimd.dma_start(swap_sb[:], swap_e[:])
            nc.gpsimd.dma_start(cos_sb[:], cos_e[:])
            nc.gpsimd.dma_start(sins_sb[:], sins_e[:])
            # x prefetch for chunks 1-3, gated on the LAST const's arrival
            # (sins cell): ungated (or gated on the same cell as the consts)
            # the 6MB of prefetch enqueues concurrently with cos/sins and
            # the rope tables crawl in behind it, stalling chunk-0's ropes
            for c in (1, 2, 3):
                for q in range(4):
                    nc.gpsimd.tensor_tensor(xs[c][q][0:1, 0:1],
                                            sins_sb[0:1, 0:1],
                                            sins_sb[0:1, 0:1], Alu.mult)
            for c in (1, 2, 3):
                for q in range(4):
                    x_dma(xs[c], c, q)
            make_identity(nc, identb[:])
            for tg in range(16):
                gen_mask(tg)

            # ---------- persistent per-head state ----------
            kt_full = [work.tile([DH, WINDOW + S], BF16, tag=f"ktf{h}", bufs=1,
                                 name=f"ktf{h}") for h in range(HPC)]
            # v rows with a ones column per 129-wide block; first 2 blocks pad
            vrow = [work.tile([128, (2 + S // 128) * VB], BF16, tag=f"vrow{h}",
                              bufs=1, name=f"vrow{h}") for h in range(HPC)]
            ckt = [work.tile([DH, NBLK], BF16, tag=f"ckt{h}", bufs=1,
                             name=f"ckt{h}") for h in range(HPC)]
            cvrow = [work.tile([NBLK, VB], BF16, tag=f"cvrow{h}", bufs=1,
                               name=f"cvrow{h}") for h in range(HPC)]
            cvt_acc = [work.tile([DH, NBLK], BF16, tag=f"cvt{h}", bufs=1,
                                 name=f"cvt{h}") for h in range(HPC)]
            for h in range(HPC):
                nc.vector.memset(kt_full[h][:, 0:WINDOW], 0.0)
                nc.vector.memset(vrow[h][:], 0.0)
                nc.vector.memset(
                    vrow[h][:].rearrange("p (b c) -> p b c", c=VB)[:, :, DH:],
                    1.0)
                nc.vector.memset(ckt[h][:], 0.0)
                nc.vector.memset(cvrow[h][:], 0.0)
                nc.vector.memset(cvrow[h][:, DH:], 1.0)
                nc.vector.memset(cvt_acc[h][:], 0.0)

            # wo tiles: loaded on the scalar queue once chunk 2's k-rope has
            # landed (rings are idle by then; earlier loads would compete
            # with the startup bulk stream)
            wo_tiles = [wopool.tile([128, 8 * 512], BF16, tag=f"wo{i}",
                                    name=f"wo{i}") for i in range(8)]

            # a2a bounce buffers: one tensor pair PER piece, in dest-major
            # TRANSPOSED layout [8 peers, 256 dims, rows-per-peer]
            a2a_in = [dram.tile([8, HPC * DH, (r1 - r0) // 8], BF16,
                                tag=f"a2ai{p}", name=f"a2ai{p}")
                      for p, (r0, r1) in enumerate(PIECES)]
            a2a_out = [dram.tile([8, HPC * DH, (r1 - r0) // 8], BF16,
                                 tag=f"a2ao{p}", name=f"a2ao{p}")
                       for p, (r0, r1) in enumerate(PIECES)]

            # bp: lhsT for the output projection. col layout per contraction
            # tile kk: [0:128]=pieces 0+1, [128:192]=piece2, [192:224]=p3,
            # [224:256]=p4
            bp_sb = xstream.tile([128, KT * 256], BF16, tag="bpt", bufs=1)

            def a2a_piece(p):
                nc.gpsimd.collective_compute(
                    "AllToAll", mybir.AluOpType.bypass,
                    replica_groups=[list(range(NCORES))],
                    ins=[a2a_in[p][:].opt()], outs=[a2a_out[p][:].opt()],
                )

            def piece_fill(p, engine):
                # bp[pp, 512i+256u+base+c] = a2a_out[p][i, 128u+pp, c]
                rp = (PIECES[p][1] - PIECES[p][0]) // 8
                base = PIECE_BASE[p]
                dst = bp_sb[:].rearrange("pp (i u c) -> pp i u c",
                                         i=8, c=256)[:, :, :, base:base + rp]
                srcv = a2a_out[p][:].rearrange("i (u pp) c -> pp i u c", u=2)
                engine.dma_start(dst, srcv)

            def proj_slab(m):
                # out rows [128m : 128m+128) of this core's 256-row share
                for n in range(4):
                    wo_sb, wo_sb2 = wo_tiles[2 * n], wo_tiles[2 * n + 1]
                    acc = ps.tile([128, 512], F32, tag=("sc", "ot")[n % 2],
                                  bufs=(3, 2)[n % 2], name=f"pacc{m}{n}")
                    for kk in range(KT):
                        wsb = wo_sb if kk < 8 else wo_sb2
                        nc.tensor.matmul(
                            acc[:],
                            bp_sb[:, 256 * kk + 128 * m:256 * kk + 128 * (m + 1)],
                            wsb[:, 512 * (kk % 8):512 * (kk % 8 + 1)],
                            start=(kk == 0), stop=(kk == KT - 1),
                        )
                    outsb = work.tile([128, 512], BF16, tag="outsb", bufs=2)
                    nc.vector.tensor_copy(outsb[:], acc[:])
                    eng = nc.sync if n % 2 == 0 else nc.scalar
                    eng.dma_start(
                        out_e[128 * m:128 * (m + 1), 512 * n:512 * (n + 1)],
                        outsb[:])

            # ---------- per-chunk work ----------
            def rope(acc, dest_ap, cols, nm):
                raw = work.tile([DH, CHUNK], BF16, tag="qraw", bufs=2,
                                name=f"raw{nm}")
                nc.scalar.copy(raw[:], acc[:])
                sw_ps = ps.tile([DH, CHUNK], F32, tag="sc", bufs=3,
                                name=f"sw{nm}")
                nc.tensor.matmul(sw_ps[:], swap_sb[:], raw[:],
                                 start=True, stop=True)
                t1 = work.tile([DH, CHUNK], BF16, tag="rope1", bufs=2,
                               name=f"t1{nm}")
                nc.vector.tensor_tensor(t1[:], raw[:], cos_sb[:, cols],
                                        Alu.mult)
                t2 = work.tile([DH, CHUNK], BF16, tag="rope2", bufs=2,
                               name=f"t2{nm}")
                nc.vector.tensor_tensor(t2[:], sw_ps[:], sins_sb[:, cols],
                                        Alu.mult)
                nc.vector.tensor_tensor(dest_ap, t1[:], t2[:], Alu.add)

            def head_setup(c, h):
                """q/k/v projections + rope + pooled/transposed kv state."""
                col0 = CHUNK * c
                cols = slice(col0, col0 + CHUNK)
                x_sb = xs[c]

                def xsl(kk):
                    t, r = kk // 4, kk % 4
                    return x_sb[t][:, CHUNK * r:CHUNK * (r + 1)]

                def one_mm(acc, w_sb, kk):
                    nc.tensor.matmul(
                        acc[:],
                        w_sb[:, 256 * kk + 128 * h:256 * kk + 128 * (h + 1)],
                        xsl(kk),
                        start=(kk == 0), stop=(kk == KT - 1),
                    )

                def acc_mm(w_sb, tag, nm):
                    acc = ps.tile([128, CHUNK], F32, tag="acc", bufs=3,
                                  name=nm)
                    for kk in range(KT):
                        one_mm(acc, w_sb, kk)
                    return acc

                if c == 0 and h == 0:
                    # chunk-0 h0 is fed at HBM rate: interleave q/k/v per x
                    # quarter so the PE consumes exactly at the delivery order
                    # and never outruns the stream
                    accs = [ps.tile([128, CHUNK], F32, tag="acc", bufs=3,
                                    name=f"acc{n}00") for n in "qkv"]
                    for qtr in range(4):
                        for acc, w_sb in zip(accs, (wq_sb, wk_sb, wv_sb)):
                            for kk in range(4 * qtr, 4 * qtr + 4):
                                one_mm(acc, w_sb, kk)
                    acc_q, acc_k0, acc_v0 = accs
                else:
                    acc_q = acc_mm(wq_sb, "accq", f"accq{c}{h}")
                # q
                q_sb = work.tile([DH, CHUNK], BF16, tag=f"qt{h}", bufs=1,
                                 name=f"qt{c}{h}")
                rope(acc_q, q_sb[:], cols, f"q{c}{h}")
                # k -> kt_full (post-rope), then pooled ck
                acc_k = acc_k0 if c == 0 and h == 0 else \
                    acc_mm(wk_sb, "acck", f"acck{c}{h}")
                kdst = kt_full[h][:, WINDOW + col0:WINDOW + col0 + CHUNK]
                rope(acc_k, kdst, cols, f"k{c}{h}")
                cks = work.tile([DH, CHUNK // RATIO], F32, tag="cks", bufs=2,
                                name=f"cks{c}{h}")
                nc.vector.tensor_reduce(
                    cks[:], kdst.rearrange("p (b r) -> p b r", r=RATIO),
                    AX.X, Alu.add)
                nc.vector.tensor_scalar_mul(
                    ckt[h][:, col0 // RATIO:(col0 + CHUNK) // RATIO],
                    cks[:], 1.0 / RATIO)
                # v: copy to bf16, pool cv, transpose to row-major
                acc_v = acc_v0 if c == 0 and h == 0 else \
                    acc_mm(wv_sb, "accv", f"accv{c}{h}")
                vt = work.tile([DH, CHUNK], BF16, tag="vt", bufs=2,
                               name=f"vt{c}{h}")
                nc.scalar.copy(vt[:], acc_v[:])
                cvs = work.tile([DH, CHUNK // RATIO], F32, tag="cks", bufs=2,
                                name=f"cvs{c}{h}")
                nc.vector.tensor_reduce(
                    cvs[:], vt[:].rearrange("p (b r) -> p b r", r=RATIO),
                    AX.X, Alu.add)
                nc.vector.tensor_scalar_mul(
                    cvt_acc[h][:, col0 // RATIO:(col0 + CHUNK) // RATIO],
                    cvs[:], 1.0 / RATIO)
                cv_ps = ps.tile([NBLK, DH], BF16, tag="ot", bufs=2,
                                name=f"cvp{c}{h}")
                nc.tensor.transpose(cv_ps[:], cvt_acc[h][:], identb[:])
                nc.scalar.copy(cvrow[h][:, 0:DH], cv_ps[:])
                vtr_ps = ps.tile([128, CHUNK], BF16, tag="ot", bufs=2,
                                 name=f"vtp{c}{h}")
                for tt in range(CHUNK // 128):
                    nc.tensor.transpose(vtr_ps[:, 128 * tt:128 * (tt + 1)],
                                        vt[:, 128 * tt:128 * (tt + 1)],
                                        identb[:])
                st0 = CHUNK // 128 * c  # first raw s-tile of this chunk
                for tt in range(CHUNK // 128):
                    nc.scalar.copy(
                        vrow[h][:, (st0 + 2 + tt) * VB:
                                (st0 + 2 + tt) * VB + DH],
                        vtr_ps[:, 128 * tt:128 * (tt + 1)])
                return q_sb

            def attn_tile(c, h, lt, q_sb):
                tg = CHUNK // 128 * c + lt   # global query tile
                qs = q_sb[:, 128 * lt:128 * (lt + 1)]
                # scores transposed [kv, q]: cols 0:128 = compressed blocks,
                # 128:512 = 3 window kv tiles
                s_ps = ps.tile([128, 512], F32, tag="sc", bufs=3,
                               name=f"sps{c}{h}{lt}")
                nc.tensor.matmul(s_ps[:, 0:128], ckt[h][:], qs,
                                 start=True, stop=True)
                for j in range(3):
                    nc.tensor.matmul(
                        s_ps[:, 128 * (j + 1):128 * (j + 2)],
                        kt_full[h][:, 128 * (tg + j):128 * (tg + j) + 128],
                        qs, start=True, stop=True)
                # p = exp(s*scale) ⊙ mask01: exp straight from PSUM (scale
                # folded into the activation), multiplicative 0/1 mask on the
                # DVE in bf16 — one hop shorter than additive-mask-then-exp,
                # and the masked entries never reach the ones-column denom
                p_r = work.tile([128, 512], BF16, tag="praw", bufs=4,
                                name=f"praw{c}{h}{lt}")
                nc.scalar.activation(p_r[:], s_ps[:], Act.Exp,
                                     bias=0.0, scale=scale)
                p_t = work.tile([128, 512], BF16, tag="psb", bufs=4,
                                name=f"psb{c}{h}{lt}")
                nc.vector.tensor_tensor(p_t[:], p_r[:],
                                        maskc_sb[:, 512 * tg:512 * (tg + 1)],
                                        Alu.mult)
                # PV: lhsT = p blocks, rhs = v rows + ones col
                o_ps = ps.tile([128, VB], F32, tag="ot", bufs=2,
                               name=f"ops{c}{h}{lt}")
                nc.tensor.matmul(o_ps[:], p_t[:, 0:128], cvrow[h][:],
                                 start=True, stop=False)
                for j in range(3):
                    w = tg - 2 + j  # raw s-tile; vrow block w+2
                    nc.tensor.matmul(
                        o_ps[:], p_t[:, 128 * (j + 1):128 * (j + 2)],
                        vrow[h][:, (w + 2) * VB:(w + 3) * VB],
                        start=False, stop=(j == 2))
                rl = work.tile([128, 1], F32, tag="stat", bufs=16,
                               name=f"rl{c}{h}{lt}")
                nc.vector.reciprocal(rl[:], o_ps[:, DH:DH + 1])
                osb = work.tile([128, DH], BF16, tag="osb", bufs=4,
                                name=f"osb{c}{h}{lt}")
                nc.vector.tensor_scalar_mul(osb[:], o_ps[:, 0:DH], rl[:])
                osbt_ps = ps.tile([128, DH], BF16, tag="ot", bufs=2,
                                  name=f"otp{c}{h}{lt}")
                nc.tensor.transpose(osbt_ps[:], osb[:], identb[:])
                osbt = work.tile([128, DH], BF16, tag="osbt", bufs=4,
                                 name=f"osbt{c}{h}{lt}")
                nc.vector.tensor_copy(osbt[:], osbt_ps[:])
                if c == 3 and h == 1:
                    # gate proj_slab(0)'s n=lt accumulation chain on this
                    # tile's output: each chain's start-matmul reads the
                    # wo cell written here, so the in-order PE stream
                    # interleaves slab-0 chains between the PV matmuls
                    # instead of hoisting the whole slab ahead of them
                    for i in (2 * lt, 2 * lt + 1):
                        nc.vector.scalar_tensor_tensor(
                            wo_tiles[i][0:1, 0:1], osb[0:1, 0:1], 0.0,
                            wo_tiles[i][0:1, 0:1], Alu.mult, Alu.add)
                if c == 1:
                    # stream one 1MB wo slab per chunk-1 tile (gated on this
                    # tile's output so the scheduler can't hoist the load)
                    i = 2 * lt + h
                    nc.vector.tensor_copy(wo_tiles[i][0:1, 0:1],
                                          osb[0:1, 0:1])
                    nc.sync.dma_start(wo_tiles[i][:], wot_e[i])
                pp = tg // 4 if tg < 12 else (3 if tg < 14 else 4)
                rp = (PIECES[pp][1] - PIECES[pp][0]) // NCORES
                row0 = 128 * tg - PIECES[pp][0]
                # one dma_start per tile (partition dim stays first on
                # both sides)
                j0, nj = row0 // rp, 128 // rp
                nc.scalar.dma_start(
                    a2a_in[pp][j0:j0 + nj, 128 * h:128 * (h + 1), :]
                    .rearrange("j d c -> d j c"),
                    osbt[:].rearrange("d (j c) -> d j c", j=nj))

            # ---------- main pipeline over s-chunks ----------
            state = {}
            for c in range(NCHUNK):
                # (wo loads are issued inside attn_tile, one 1MB slab per
                # chunk-1 attention tile, so the 8.4MB never lands as one
                # burst: a burst saturates the chip-shared HBM and queues
                # ahead of the a2a_in writes in the ring FIFOs, which delays
                # every core's arrival at the piece collectives)
                q_sbs = [head_setup(c, h) for h in range(HPC)]
                for lt in range(CHUNK // 128):
                    for h in range(HPC):
                        attn_tile(c, h, lt, q_sbs[h])
                    if c == 3 and lt == 1:
                        a2a_piece(3)   # rows 1536:1792 done mid-chunk
                # ---------- overlapped collectives + projection ----------
                # fills live on the gpsimd queue, emitted just before the
                # NEXT collective: the gpsimd sequencer blocks on each
                # collective's completion anyway, so fill p triggers the
                # moment piece p is done without blocking anything else
                if c == 0:
                    a2a_piece(0)
                elif c == 1:
                    a2a_piece(1)
                elif c == 2:
                    a2a_piece(2)
                    piece_fill(0, nc.sync)
                    piece_fill(1, nc.sync)
                elif c == 3:
                    a2a_piece(4)       # tail piece (rows 1792:2048)
                    piece_fill(2, nc.sync)
                    piece_fill(3, nc.sync)
                    piece_fill(4, nc.sync)
                    proj_slab(0)       # interleaved with chunk-3 PVs
                    proj_slab(1)
    return nc


def _host_inputs(x, wq, wk, wv, wo):
    """Build per-core input maps (numpy)."""
    import ml_dtypes
    BF = ml_dtypes.bfloat16
    xT = x.reshape(S, D).T.astype(BF)          # [D, S]
    xtile = np.ascontiguousarray(
        xT.reshape(KT, 128, NCHUNK, CHUNK).transpose(2, 1, 0, 3)
        .reshape(NCHUNK, 128, KT * CHUNK))
    woT = wo.T.astype(BF)                      # [D, D]
    wotile = np.ascontiguousarray(
        woT.reshape(2, 8, 128, 4, 512).transpose(3, 0, 2, 1, 4)
        .reshape(8, 128, 8 * 512))

    def wtile(w, rows):
        wT = w[rows, :].T.astype(BF)           # [D, 256]
        return np.ascontiguousarray(
            wT.reshape(KT, 128, 256).transpose(1, 0, 2).reshape(128, KT * 256))

    inv = 1.0 / (ROPE_BASE ** (np.arange(0, DH, 2, dtype=np.float32) / DH))
    theta = np.outer(np.arange(S, dtype=np.float32), inv)  # [S, 64]
    cos = np.cos(theta).T  # [64, S]
    sin = np.sin(theta).T
    COS = np.empty((DH, S), np.float32)
    SINS = np.empty((DH, S), np.float32)
    COS[0::2] = cos
    COS[1::2] = cos
    SINS[0::2] = -sin
    SINS[1::2] = sin

    SWAP = np.zeros((DH, DH), np.float32)
    for t in range(DH // 2):
        SWAP[2 * t + 1, 2 * t] = 1.0
        SWAP[2 * t, 2 * t + 1] = 1.0

    in_maps = []
    for cid in range(NCORES):
        rows = slice(256 * cid, 256 * (cid + 1))
        in_maps.append({
            "xt": xtile,
            "wqt": wtile(wq, rows),
            "wkt": wtile(wk, rows),
            "wvt": wtile(wv, rows),
            "wot": wotile,
            "cos": COS.astype(BF),
            "sins": SINS.astype(BF),
            "swapm": SWAP.astype(BF),
        })
    return in_maps


_CACHE = {}
LAST_EXEC_NS = None
LAST_RES = None


def kernel(x, wq, wk, wv, wo):
    _setup_ntff_hook()
    from concourse.bass_utils import run_bass_kernel_spmd

    if "nc" not in _CACHE:
        ncb = build()
        if not ncb.is_finalized():
            ncb.finalize()
        _CACHE["nc"] = ncb
    ncb = _CACHE["nc"]

    in_maps = _host_inputs(np.asarray(x), np.asarray(wq), np.asarray(wk),
                           np.asarray(wv), np.asarray(wo))
    trace = bool(os.environ.get("KERNEL_TRACE"))
    res = run_bass_kernel_spmd(ncb, in_maps, list(range(NCORES)), trace=trace)
    globals()["LAST_EXEC_NS"] = res.exec_time_ns
    globals()["LAST_RES"] = res
    # Each core's 256 out rows: piece p contributes rp=(r1-r0)/8 local
    # rows mapping to global rows r0 + rp*cid + l.
    out = np.empty((S, D), np.float32)
    for cid in range(NCORES):
        o = np.asarray(res.results[cid]["out"], dtype=np.float32)
        off = 0
        for p, (r0, r1) in enumerate(PIECES):
            rp = (r1 - r0) // NCORES
            out[r0 + rp * cid:r0 + rp * (cid + 1)] = o[off:off + rp]
            off += rp
    return out.reshape(1, S, D)


if __name__ == "__main__":
    rng = np.random.default_rng(0)
    x = rng.standard_normal((1, S, D), dtype=np.float32)
    wq = rng.standard_normal((D, D), dtype=np.float32) * D ** -0.5
    wk = rng.standard_normal((D, D), dtype=np.float32) * D ** -0.5
    wv = rng.standard_normal((D, D), dtype=np.float32) * D ** -0.5
    wo = rng.standard_normal((D, D), dtype=np.float32) * D ** -0.5
    out = kernel(x=x, wq=wq, wk=wk, wv=wv, wo=wo)
    print("out", out.shape, out.dtype, np.abs(out).mean())


# revision 39
# speedup vs baseline: 1.0833x; 1.0240x over previous
"""Distributed TRN2 Bass kernel for NSA-style sparse attention.

Problem: b=1, s=2048, d=2048, 16 heads x 128 dim, f32.
  q/k/v = x @ w{q,k,v}.T ; interleaved RoPE on q,k ;
  compressed KV = mean-pool of 16 post-RoPE tokens ;
  joint softmax over [causal compressed blocks ; 256-token sliding window] ;
  out = (p @ [cv;v]) @ wo.T

Sharding: 2 heads per core (column-parallel wq/wk/wv), x replicated.
Output projection via AllToAll, chunked into 5 row pieces; collectives
and the wo matmuls overlap the attention of later chunks. Chunk 3 runs
both heads' projections first, then attention tiles row-major across
heads so piece 3 (rows 1536:1792) posts mid-chunk and piece 4 right
after the last tile; proj_slab(0) then fills the PE while the tail
collectives run.

Attention computes scores transposed ([kv, q]) so the exp output feeds
the PV matmul directly as the stationary operand. The compressed-block
scores land in the same [128,512] PSUM tile as the window scores, so a
single scalar_tensor_tensor applies scale+mask from a combined table.
The softmax denominator comes from a ones-column appended to v.

Startup DMAs are issued fine-grained in exact consumption order
(wq quarter, x0 halves, ... wk, wv) with the rope/mask constants gated
behind x0's third quarter so they don't steal critical-path bandwidth.
wo streams in during chunk 2 (gated on chunk-2 k-rope) when the rings
are idle.

Precision: matmul operands bf16 (f32 PSUM accumulation), softmax in
f32; rope multiplies in bf16.
"""
import sys, os, types

sys.path.insert(0, "/opt/trn_rl_repo")
import numpy as np

S = 2048        # sequence length
D = 2048        # model dim
H = 16          # heads
DH = 128        # head dim
RATIO = 16      # compress ratio
WINDOW = 256    # sliding window
NBLK = S // RATIO          # 128 compressed blocks
ROPE_BASE = 10000.0
NCORES = 8
HPC = H // NCORES          # 2 heads per core
CHUNK = 512                # s-columns per pipeline step
NCHUNK = S // CHUNK        # 4
KT = D // 128              # 16 contraction tiles
NEG = -1e30
VB = DH + 1                # v row block width (ones column at DH)

# A2A pieces: (row0, row1) over the s axis; per-core slab column base in bp
PIECES = [(0, 512), (512, 1024), (1024, 1536), (1536, 1792), (1792, 2048)]
PIECE_BASE = [0, 64, 128, 192, 224]


def _setup_ntff_hook():
    try:
        import antenv
        if "antenv.axon_hooks" not in sys.modules:
            m = types.ModuleType("antenv.axon_hooks")
            m._hook = None
            m.set_axon_ntff_profile_hook = lambda h: setattr(m, "_hook", h)
            m.get_axon_ntff_profile_hook = lambda: m._hook
            sys.modules["antenv.axon_hooks"] = m
            antenv.axon_hooks = m
        if "/root/.axon_site" not in sys.path:
            sys.path.insert(0, "/root/.axon_site")
        from trn_agent_boot.trn_boot import _ntff_profile_via_ctypes
        hook = _ntff_profile_via_ctypes("/opt/axon/libaxon_pjrt.so")
        sys.modules["antenv.axon_hooks"].set_axon_ntff_profile_hook(hook)
    except Exception:
        pass


def build():
    import concourse.bass as bass
    import concourse.mybir as mybir
    from concourse import bacc, tile
    from concourse.masks import make_identity

    F32 = mybir.dt.float32
    BF16 = mybir.dt.bfloat16
    Alu = mybir.AluOpType
    Act = mybir.ActivationFunctionType
    AX = mybir.AxisListType

    nc = bacc.Bacc(None, target_bir_lowering=False, debug=False)

    xt_e = nc.declare_dram_parameter("xt", [NCHUNK, 128, KT * CHUNK], BF16,
                                     isOutput=False)
    wqt_e = nc.declare_dram_parameter("wqt", [128, KT * 256], BF16, isOutput=False)
    wkt_e = nc.declare_dram_parameter("wkt", [128, KT * 256], BF16, isOutput=False)
    wvt_e = nc.declare_dram_parameter("wvt", [128, KT * 256], BF16, isOutput=False)
    wot_e = nc.declare_dram_parameter("wot", [8, 128, 8 * 512], BF16,
                                      isOutput=False)
    cos_e = nc.declare_dram_parameter("cos", [DH, S], BF16, isOutput=False)
    sins_e = nc.declare_dram_parameter("sins", [DH, S], BF16, isOutput=False)
    swap_e = nc.declare_dram_parameter("swapm", [DH, DH], BF16, isOutput=False)
    out_e = nc.declare_dram_parameter("out", [S // NCORES, D], BF16,
                                     isOutput=True)

    scale = float(DH) ** -0.5
    Q = 1024   # columns in a quarter of a projection weight tile

    with tile.TileContext(nc) as tc:
        with (
            tc.tile_pool(name="const", bufs=1) as constp,
            tc.tile_pool(name="wpool", bufs=1) as wpool,
            tc.tile_pool(name="wopool", bufs=1) as wopool,
            tc.tile_pool(name="xstream", bufs=2) as xstream,
            tc.tile_pool(name="work", bufs=2) as work,
            tc.tile_pool(name="ps", bufs=2, space="PSUM") as ps,
            tc.tile_pool(name="dram", bufs=1, space="DRAM") as dram,
        ):
            # ---------- critical startup stream (sync queue) ----------------
            # Exact consumption order for chunk-0 head-0's sequential q/k/v
            # accumulations: wq quarter q feeds matmuls kk=4q..4q+3 together
            # with x tile q; then wk, wv halves; then x for chunks 1-3.
            wq_sb = wpool.tile([128, KT * 256], BF16, tag="wq")
            wk_sb = wpool.tile([128, KT * 256], BF16, tag="wk")
            wv_sb = wpool.tile([128, KT * 256], BF16, tag="wv")

            def x_tiles(c):
                return [xstream.tile([128, 4 * CHUNK], BF16, tag=f"xq{q}",
                                     name=f"x{c}q{q}") for q in range(4)]

            def x_dma(ts, c, q):
                nc.sync.dma_start(ts[q][:], xt_e[c][:, 4 * CHUNK * q:
                                                    4 * CHUNK * (q + 1)])

            xs = {0: x_tiles(0), 1: x_tiles(1)}
            for q in range(4):
                nc.sync.dma_start(wq_sb[:, Q * q:Q * (q + 1)],
                                  wqt_e[:, Q * q:Q * (q + 1)])
                nc.sync.dma_start(xs[0][q][:, 0:Q], xt_e[0][:, 2048 * q:
                                                            2048 * q + Q])
                nc.sync.dma_start(xs[0][q][:, Q:2048], xt_e[0][:, 2048 * q + Q:
                                                               2048 * (q + 1)])
                nc.sync.dma_start(wk_sb[:, Q * q:Q * (q + 1)],
                                  wkt_e[:, Q * q:Q * (q + 1)])
                nc.sync.dma_start(wv_sb[:, Q * q:Q * (q + 1)],
                                  wvt_e[:, Q * q:Q * (q + 1)])
            for c in (2, 3):
                xs[c] = x_tiles(c)

            # ---------- constants (gpsimd queue), gated behind x0q2 ---------
            swap_sb = constp.tile([DH, DH], BF16, tag="swap")
            cos_sb = constp.tile([DH, S], BF16, tag="cos")
            sins_sb = constp.tile([DH, S], BF16, tag="sins")
            identb = constp.tile([128, 128], BF16, tag="identb")
            maskc_sb = constp.tile([128, 16 * 512], BF16, tag="maskc")
            gate = constp.tile([1, 1], BF16, tag="gate")

            def gen_mask(tg):
                # 0/1 mask for query tile tg, generated on the idle gpsimd
                # engine (saves 2MB of HBM from the startup window):
                # cols 0:128 comp blocks vis iff blk < 8tg + (r+1)//16;
                # window tiles w=tg-2+j: j=0 anti-triangle, j=1 full,
                # j=2 causal triangle; padded tiles (w<0) fully masked
                base = 512 * tg
                comp = maskc_sb[:, base:base + 128]
                nc.gpsimd.memset(comp, 1.0)
                cv = comp.rearrange("p (rb rr) -> p rb rr", rr=16)
                # (r+1)//16 = rb + (rr==15): two disjoint affine selects
                # (only is_gt/is_ge lower on gpsimd, so conditions are
                # written as rb + 8tg(+1) - blk > 0)
                nc.gpsimd.affine_select(
                    out=cv[:, :, 0:15], in_=cv[:, :, 0:15],
                    compare_op=Alu.is_gt, fill=0.0, base=8 * tg,
                    pattern=[[1, 8], [0, 15]], channel_multiplier=-1)
                nc.gpsimd.affine_select(
                    out=cv[:, :, 15:16], in_=cv[:, :, 15:16],
                    compare_op=Alu.is_gt, fill=0.0, base=8 * tg + 1,
                    pattern=[[1, 8], [0, 1]], channel_multiplier=-1)
                win = maskc_sb[:, base + 128:base + 512]
                if tg == 0:
                    nc.gpsimd.memset(win[:, 0:256], 0.0)
                    nc.gpsimd.memset(win[:, 256:384], 1.0)
                else:
                    nc.gpsimd.memset(win, 1.0)
                    if tg == 1:
                        nc.gpsimd.memset(win[:, 0:128], 0.0)
                    else:
                        # j=0: vis iff p > r
                        nc.gpsimd.affine_select(
                            out=win[:, 0:128], in_=win[:, 0:128],
                            compare_op=Alu.is_gt, fill=0.0, base=0,
                            pattern=[[-1, 128]], channel_multiplier=1)
                # j=2: vis iff p <= r, i.e. r - p >= 0
                nc.gpsimd.affine_select(
                    out=win[:, 256:384], in_=win[:, 256:384],
                    compare_op=Alu.is_ge, fill=0.0, base=0,
                    pattern=[[1, 128]], channel_multiplier=-1)

            # RAW gate: rope tables enqueue once x0q1 landed so they don't
            # steal ring bandwidth from the critical first quarters; gate and
            # const DMAs go FIRST on the gpsimd queue so nothing delays them
            nc.gpsimd.tensor_copy(gate[0:1, 0:1], xs[0][1][0:1, 0:1])
            for t in (swap_sb, cos_sb, sins_sb):
                nc.gpsimd.tensor_tensor(t[0:1, 0:1], gate[0:1, 0:1],
                                        gate[0:1, 0:1], Alu.mult)
            nc.gpsimd.dma_start(swap_sb[:], swap_e[:])
            nc.gpsimd.dma_start(cos_sb[:], cos_e[:])
            nc.gpsimd.dma_start(sins_sb[:], sins_e[:])
            # x prefetch for chunks 1-3, gated on the LAST const's arrival
            # (sins cell): ungated (or gated on the same cell as the consts)
            # the 6MB of prefetch enqueues concurrently with cos/sins and
            # the rope tables crawl in behind it, stalling chunk-0's ropes
            for c in (1, 2, 3):
                for q in range(4):
                    nc.gpsimd.tensor_tensor(xs[c][q][0:1, 0:1],
                                            sins_sb[0:1, 0:1],
                                            sins_sb[0:1, 0:1], Alu.mult)
            for c in (1, 2, 3):
                for q in range(4):
                    x_dma(xs[c], c, q)
            make_identity(nc, identb[:])
            for tg in range(16):
                gen_mask(tg)

            # ---------- persistent per-head state ----------
            kt_full = [work.tile([DH, WINDOW + S], BF16, tag=f"ktf{h}", bufs=1,
                                 name=f"ktf{h}") for h in range(HPC)]
            # v rows with a ones column per 129-wide block; first 2 blocks pad
            vrow = [work.tile([128, (2 + S // 128) * VB], BF16, tag=f"vrow{h}",
                              bufs=1, name=f"vrow{h}") for h in range(HPC)]
            ckt = [work.tile([DH, NBLK], BF16, tag=f"ckt{h}", bufs=1,
                             name=f"ckt{h}") for h in range(HPC)]
            cvrow = [work.tile([NBLK, VB], BF16, tag=f"cvrow{h}", bufs=1,
                               name=f"cvrow{h}") for h in range(HPC)]
            cvt_acc = [work.tile([DH, NBLK], BF16, tag=f"cvt{h}", bufs=1,
                                 name=f"cvt{h}") for h in range(HPC)]
            for h in range(HPC):
                nc.vector.memset(kt_full[h][:, 0:WINDOW], 0.0)
                nc.vector.memset(vrow[h][:], 0.0)
                nc.vector.memset(
                    vrow[h][:].rearrange("p (b c) -> p b c", c=VB)[:, :, DH:],
                    1.0)
                nc.vector.memset(ckt[h][:], 0.0)
                nc.vector.memset(cvrow[h][:], 0.0)
                nc.vector.memset(cvrow[h][:, DH:], 1.0)
                nc.vector.memset(cvt_acc[h][:], 0.0)

            # wo tiles: loaded on the scalar queue once chunk 2's k-rope has
            # landed (rings are idle by then; earlier loads would compete
            # with the startup bulk stream)
            wo_tiles = [wopool.tile([128, 8 * 512], BF16, tag=f"wo{i}",
                                    name=f"wo{i}") for i in range(8)]

            # a2a bounce buffers: one tensor pair PER piece, in dest-major
            # TRANSPOSED layout [8 peers, 256 dims, rows-per-peer]
            a2a_in = [dram.tile([8, HPC * DH, (r1 - r0) // 8], BF16,
                                tag=f"a2ai{p}", name=f"a2ai{p}")
                      for p, (r0, r1) in enumerate(PIECES)]
            a2a_out = [dram.tile([8, HPC * DH, (r1 - r0) // 8], BF16,
                                 tag=f"a2ao{p}", name=f"a2ao{p}")
                       for p, (r0, r1) in enumerate(PIECES)]

            # bp: lhsT for the output projection. col layout per contraction
            # tile kk: [0:128]=pieces 0+1, [128:192]=piece2, [192:224]=p3,
            # [224:256]=p4
            bp_sb = xstream.tile([128, KT * 256], BF16, tag="bpt", bufs=1)

            def a2a_piece(p):
                nc.gpsimd.collective_compute(
                    "AllToAll", mybir.AluOpType.bypass,
                    replica_groups=[list(range(NCORES))],
                    ins=[a2a_in[p][:].opt()], outs=[a2a_out[p][:].opt()],
                )

            def piece_fill(p, engine):
                # bp[pp, 512i+256u+base+c] = a2a_out[p][i, 128u+pp, c]
                rp = (PIECES[p][1] - PIECES[p][0]) // 8
                base = PIECE_BASE[p]
                dst = bp_sb[:].rearrange("pp (i u c) -> pp i u c",
                                         i=8, c=256)[:, :, :, base:base + rp]
                srcv = a2a_out[p][:].rearrange("i (u pp) c -> pp i u c", u=2)
                engine.dma_start(dst, srcv)

            def proj_slab(m):
                # out rows [128m : 128m+128) of this core's 256-row share
                for n in range(4):
                    wo_sb, wo_sb2 = wo_tiles[2 * n], wo_tiles[2 * n + 1]
                    acc = ps.tile([128, 512], F32, tag=("sc", "ot")[n % 2],
                                  bufs=(3, 2)[n % 2], name=f"pacc{m}{n}")
                    for kk in range(KT):
                        wsb = wo_sb if kk < 8 else wo_sb2
                        nc.tensor.matmul(
                            acc[:],
                            bp_sb[:, 256 * kk + 128 * m:256 * kk + 128 * (m + 1)],
                            wsb[:, 512 * (kk % 8):512 * (kk % 8 + 1)],
                            start=(kk == 0), stop=(kk == KT - 1),
                        )
                    outsb = work.tile([128, 512], BF16, tag="outsb", bufs=2)
                    nc.vector.tensor_copy(outsb[:], acc[:])
                    eng = nc.sync if n % 2 == 0 else nc.scalar
                    eng.dma_start(
                        out_e[128 * m:128 * (m + 1), 512 * n:512 * (n + 1)],
                        outsb[:])

            # ---------- per-chunk work ----------
            def rope(acc, dest_ap, cols, nm):
                raw = work.tile([DH, CHUNK], BF16, tag="qraw", bufs=2,
                                name=f"raw{nm}")
                nc.scalar.copy(raw[:], acc[:])
                sw_ps = ps.tile([DH, CHUNK], F32, tag="sc", bufs=3,
                                name=f"sw{nm}")
                nc.tensor.matmul(sw_ps[:], swap_sb[:], raw[:],
                                 start=True, stop=True)
                t1 = work.tile([DH, CHUNK], BF16, tag="rope1", bufs=2,
                               name=f"t1{nm}")
                nc.vector.tensor_tensor(t1[:], raw[:], cos_sb[:, cols],
                                        Alu.mult)
                t2 = work.tile([DH, CHUNK], BF16, tag="rope2", bufs=2,
                               name=f"t2{nm}")
                nc.vector.tensor_tensor(t2[:], sw_ps[:], sins_sb[:, cols],
                                        Alu.mult)
                nc.vector.tensor_tensor(dest_ap, t1[:], t2[:], Alu.add)

            def head_setup(c, h):
                """q/k/v projections + rope + pooled/transposed kv state."""
                col0 = CHUNK * c
                cols = slice(col0, col0 + CHUNK)
                x_sb = xs[c]

                def xsl(kk):
                    t, r = kk // 4, kk % 4
                    return x_sb[t][:, CHUNK * r:CHUNK * (r + 1)]

                def one_mm(acc, w_sb, kk):
                    nc.tensor.matmul(
                        acc[:],
                        w_sb[:, 256 * kk + 128 * h:256 * kk + 128 * (h + 1)],
                        xsl(kk),
                        start=(kk == 0), stop=(kk == KT - 1),
                    )

                def acc_mm(w_sb, tag, nm):
                    acc = ps.tile([128, CHUNK], F32, tag="acc", bufs=3,
                                  name=nm)
                    for kk in range(KT):
                        one_mm(acc, w_sb, kk)
                    return acc

                if c == 0 and h == 0:
                    # chunk-0 h0 is fed at HBM rate: interleave q/k/v per x
                    # quarter so the PE consumes exactly at the delivery order
                    # and never outruns the stream
                    accs = [ps.tile([128, CHUNK], F32, tag="acc", bufs=3,
                                    name=f"acc{n}00") for n in "qkv"]
                    for qtr in range(4):
                        for acc, w_sb in zip(accs, (wq_sb, wk_sb, wv_sb)):
                            for kk in range(4 * qtr, 4 * qtr + 4):
                                one_mm(acc, w_sb, kk)
                    acc_q, acc_k0, acc_v0 = accs
                else:
                    acc_q = acc_mm(wq_sb, "accq", f"accq{c}{h}")
                # q
                q_sb = work.tile([DH, CHUNK], BF16, tag=f"qt{h}", bufs=1,
                                 name=f"qt{c}{h}")
                rope(acc_q, q_sb[:], cols, f"q{c}{h}")
                # k -> kt_full (post-rope), then pooled ck
                acc_k = acc_k0 if c == 0 and h == 0 else \
                    acc_mm(wk_sb, "acck", f"acck{c}{h}")
                kdst = kt_full[h][:, WINDOW + col0:WINDOW + col0 + CHUNK]
                rope(acc_k, kdst, cols, f"k{c}{h}")
                cks = work.tile([DH, CHUNK // RATIO], F32, tag="cks", bufs=2,
                                name=f"cks{c}{h}")
                nc.vector.tensor_reduce(
                    cks[:], kdst.rearrange("p (b r) -> p b r", r=RATIO),
                    AX.X, Alu.add)
                nc.vector.tensor_scalar_mul(
                    ckt[h][:, col0 // RATIO:(col0 + CHUNK) // RATIO],
                    cks[:], 1.0 / RATIO)
                # v: copy to bf16, pool cv, transpose to row-major
                acc_v = acc_v0 if c == 0 and h == 0 else \
                    acc_mm(wv_sb, "accv", f"accv{c}{h}")
                vt = work.tile([DH, CHUNK], BF16, tag="vt", bufs=2,
                               name=f"vt{c}{h}")
                nc.scalar.copy(vt[:], acc_v[:])
                cvs = work.tile([DH, CHUNK // RATIO], F32, tag="cks", bufs=2,
                                name=f"cvs{c}{h}")
                nc.vector.tensor_reduce(
                    cvs[:], vt[:].rearrange("p (b r) -> p b r", r=RATIO),
                    AX.X, Alu.add)
                nc.vector.tensor_scalar_mul(
                    cvt_acc[h][:, col0 // RATIO:(col0 + CHUNK) // RATIO],
                    cvs[:], 1.0 / RATIO)
                cv_ps = ps.tile([NBLK, DH], BF16, tag="ot", bufs=2,
                                name=f"cvp{c}{h}")
                nc.tensor.transpose(cv_ps[:], cvt_acc[h][:], identb[:])
                nc.scalar.copy(cvrow[h][:, 0:DH], cv_ps[:])
                vtr_ps = ps.tile([128, CHUNK], BF16, tag="ot", bufs=2,
                                 name=f"vtp{c}{h}")
                for tt in range(CHUNK // 128):
                    nc.tensor.transpose(vtr_ps[:, 128 * tt:128 * (tt + 1)],
                                        vt[:, 128 * tt:128 * (tt + 1)],
                                        identb[:])
                st0 = CHUNK // 128 * c  # first raw s-tile of this chunk
                for tt in range(CHUNK // 128):
                    nc.scalar.copy(
                        vrow[h][:, (st0 + 2 + tt) * VB:
                                (st0 + 2 + tt) * VB + DH],
                        vtr_ps[:, 128 * tt:128 * (tt + 1)])
                return q_sb

            def attn_tile(c, h, lt, q_sb):
                tg = CHUNK // 128 * c + lt   # global query tile
                qs = q_sb[:, 128 * lt:128 * (lt + 1)]
                # scores transposed [kv, q]: cols 0:128 = compressed blocks,
                # 128:512 = 3 window kv tiles
                s_ps = ps.tile([128, 512], F32, tag="sc", bufs=3,
                               name=f"sps{c}{h}{lt}")
                nc.tensor.matmul(s_ps[:, 0:128], ckt[h][:], qs,
                                 start=True, stop=True)
                for j in range(3):
                    nc.tensor.matmul(
                        s_ps[:, 128 * (j + 1):128 * (j + 2)],
                        kt_full[h][:, 128 * (tg + j):128 * (tg + j) + 128],
                        qs, start=True, stop=True)
                # p = exp(s*scale) ⊙ mask01: exp straight from PSUM (scale
                # folded into the activation), multiplicative 0/1 mask on the
                # DVE in bf16 — one hop shorter than additive-mask-then-exp,
                # and the masked entries never reach the ones-column denom
                p_r = work.tile([128, 512], BF16, tag="praw", bufs=4,
                                name=f"praw{c}{h}{lt}")
                nc.scalar.activation(p_r[:], s_ps[:], Act.Exp,
                                     bias=0.0, scale=scale)
                p_t = work.tile([128, 512], BF16, tag="psb", bufs=4,
                                name=f"psb{c}{h}{lt}")
                nc.vector.tensor_tensor(p_t[:], p_r[:],
                                        maskc_sb[:, 512 * tg:512 * (tg + 1)],
                                        Alu.mult)
                # PV: lhsT = p blocks, rhs = v rows + ones col
                o_ps = ps.tile([128, VB], F32, tag="ot", bufs=2,
                               name=f"ops{c}{h}{lt}")
                nc.tensor.matmul(o_ps[:], p_t[:, 0:128], cvrow[h][:],
                                 start=True, stop=False)
                for j in range(3):
                    w = tg - 2 + j  # raw s-tile; vrow block w+2
                    nc.tensor.matmul(
                        o_ps[:], p_t[:, 128 * (j + 1):128 * (j + 2)],
                        vrow[h][:, (w + 2) * VB:(w + 3) * VB],
                        start=False, stop=(j == 2))
                rl = work.tile([128, 1], F32, tag="stat", bufs=16,
                               name=f"rl{c}{h}{lt}")
                nc.vector.reciprocal(rl[:], o_ps[:, DH:DH + 1])
                osb = work.tile([128, DH], BF16, tag="osb", bufs=4,
                                name=f"osb{c}{h}{lt}")
                nc.vector.tensor_scalar_mul(osb[:], o_ps[:, 0:DH], rl[:])
                osbt_ps = ps.tile([128, DH], BF16, tag="ot", bufs=2,
                                  name=f"otp{c}{h}{lt}")
                nc.tensor.transpose(osbt_ps[:], osb[:], identb[:])
                osbt = work.tile([128, DH], BF16, tag="osbt", bufs=4,
                                 name=f"osbt{c}{h}{lt}")
                nc.vector.tensor_copy(osbt[:], osbt_ps[:])
                if c == 3 and h == 1:
                    # gate proj_slab(0)'s n=lt accumulation chain on this
                    # tile's output: each chain's start-matmul reads the
                    # wo cell written here, so the in-order PE stream
                    # interleaves slab-0 chains between the PV matmuls
                    # instead of hoisting the whole slab ahead of them
                    for i in (2 * lt, 2 * lt + 1):
                        nc.vector.scalar_tensor_tensor(
                            wo_tiles[i][0:1, 0:1], osb[0:1, 0:1], 0.0,
                            wo_tiles[i][0:1, 0:1], Alu.mult, Alu.add)
                if c == 1:
                    # stream one 1MB wo slab per chunk-1 tile (gated on this
                    # tile's output so the scheduler can't hoist the load)
                    i = 2 * lt + h
                    nc.vector.tensor_copy(wo_tiles[i][0:1, 0:1],
                                          osb[0:1, 0:1])
                    nc.sync.dma_start(wo_tiles[i][:], wot_e[i])
                pp = tg // 4 if tg < 12 else (3 if tg < 14 else 4)
                rp = (PIECES[pp][1] - PIECES[pp][0]) // NCORES
                row0 = 128 * tg - PIECES[pp][0]
                # one dma_start per tile (partition dim stays first on
                # both sides)
                j0, nj = row0 // rp, 128 // rp
                nc.scalar.dma_start(
                    a2a_in[pp][j0:j0 + nj, 128 * h:128 * (h + 1), :]
                    .rearrange("j d c -> d j c"),
                    osbt[:].rearrange("d (j c) -> d j c", j=nj))

            # ---------- main pipeline over s-chunks ----------
            state = {}
            for c in range(NCHUNK):
                # (wo loads are issued inside attn_tile, one 1MB slab per
                # chunk-1 attention tile, so the 8.4MB never lands as one
                # burst: a burst saturates the chip-shared HBM and queues
                # ahead of the a2a_in writes in the ring FIFOs, which delays
                # every core's arrival at the piece collectives)
                q_sbs = [head_setup(c, h) for h in range(HPC)]
                for lt in range(CHUNK // 128):
                    for h in range(HPC):
                        attn_tile(c, h, lt, q_sbs[h])
                    if c == 3 and lt == 1:
                        a2a_piece(3)   # rows 1536:1792 done mid-chunk
                # ---------- overlapped collectives + projection ----------
                # fills live on the gpsimd queue, emitted just before the
                # NEXT collective: the gpsimd sequencer blocks on each
                # collective's completion anyway, so fill p triggers the
                # moment piece p is done without blocking anything else
                if c == 0:
                    a2a_piece(0)
                elif c == 1:
                    a2a_piece(1)
                elif c == 2:
                    a2a_piece(2)
                    piece_fill(0, nc.sync)
                    piece_fill(1, nc.sync)
                elif c == 3:
                    a2a_piece(4)       # tail piece (rows 1792:2048)
                    piece_fill(2, nc.sync)
                    piece_fill(3, nc.sync)
                    piece_fill(4, nc.sync)
                    proj_slab(0)       # interleaved with chunk-3 PVs
                    proj_slab(1)
    return nc


def _host_inputs(x, wq, wk, wv, wo):
    """Build per-core input maps (numpy)."""
    import ml_dtypes
    BF = ml_dtypes.bfloat16
    xT = x.reshape(S, D).T.astype(BF)          # [D, S]
    xtile = np.ascontiguousarray(
        xT.reshape(KT, 128, NCHUNK, CHUNK).transpose(2, 1, 0, 3)
        .reshape(NCHUNK, 128, KT * CHUNK))
    woT = wo.T.astype(BF)                      # [D, D]
    wotile = np.ascontiguousarray(
        woT.reshape(2, 8, 128, 4, 512).transpose(3, 0, 2, 1, 4)
        .reshape(8, 128, 8 * 512))

    def wtile(w, rows):
        wT = w[rows, :].T.astype(BF)           # [D, 256]
        return np.ascontiguousarray(
            wT.reshape(KT, 128, 256).transpose(1, 0, 2).reshape(128, KT * 256))

    inv = 1.0 / (ROPE_BASE ** (np.arange(0, DH, 2, dtype=np.float32) / DH))
    theta = np.outer(np.arange(S, dtype=np.float32), inv)  # [S, 64]
    cos = np.cos(theta).T  # [64, S]
    sin = np.sin(theta).T
    COS = np.empty((DH, S), np.float32)
    SINS = np.empty((DH, S), np.float32)
    COS[0::2] = cos
    COS[1::2] = cos
    SINS[0::2] = -sin
    SINS[1::2] = sin

    SWAP = np.zeros((DH, DH), np.float32)
    for t in range(DH // 2):
        SWAP[2 * t + 1, 2 * t] = 1.0
        SWAP[2 * t, 2 * t + 1] = 1.0

    in_maps = []
    for cid in range(NCORES):
        rows = slice(256 * cid, 256 * (cid + 1))
        in_maps.append({
            "xt": xtile,
            "wqt": wtile(wq, rows),
            "wkt": wtile(wk, rows),
            "wvt": wtile(wv, rows),
            "wot": wotile,
            "cos": COS.astype(BF),
            "sins": SINS.astype(BF),
            "swapm": SWAP.astype(BF),
        })
    return in_maps


_CACHE = {}
LAST_EXEC_NS = None
LAST_RES = None


def kernel(x, wq, wk, wv, wo):
    _setup_ntff_hook()
    from concourse.bass_utils import run_bass_kernel_spmd

    if "nc" not in _CACHE:
        ncb = build()
        if not ncb.is_finalized():
            ncb.finalize()
        _CACHE["nc"] = ncb
    ncb = _CACHE["nc"]

    in_maps = _host_inputs(np.asarray(x), np.asarray(wq), np.asarray(wk),
                           np.asarray(wv), np.asarray(wo))
    trace = bool(os.environ.get("KERNEL_TRACE"))
    res = run_bass_kernel_spmd(ncb, in_maps, list(range(NCORES)), trace=trace,
                               trace_cores=[0, 4] if os.environ.get('TRACE2') else None)
    globals()["LAST_EXEC_NS"] = res.exec_time_ns
    globals()["LAST_RES"] = res
    # Each core's 256 out rows: piece p contributes rp=(r1-r0)/8 local
    # rows mapping to global rows r0 + rp*cid + l.
    out = np.empty((S, D), np.float32)
    for cid in range(NCORES):
        o = np.asarray(res.results[cid]["out"], dtype=np.float32)
        off = 0
        for p, (r0, r1) in enumerate(PIECES):
            rp = (r1 - r0) // NCORES
            out[r0 + rp * cid:r0 + rp * (cid + 1)] = o[off:off + rp]
            off += rp
    return out.reshape(1, S, D)


if __name__ == "__main__":
    rng = np.random.default_rng(0)
    x = rng.standard_normal((1, S, D), dtype=np.float32)
    wq = rng.standard_normal((D, D), dtype=np.float32) * D ** -0.5
    wk = rng.standard_normal((D, D), dtype=np.float32) * D ** -0.5
    wv = rng.standard_normal((D, D), dtype=np.float32) * D ** -0.5
    wo = rng.standard_normal((D, D), dtype=np.float32) * D ** -0.5
    out = kernel(x=x, wq=wq, wk=wk, wv=wv, wo=wo)
    print("out", out.shape, out.dtype, np.abs(out).mean())
